# revision 1
# baseline (speedup 1.0000x reference)
"""
AngularPenaltySMLoss ("cosface"-style additive-angular-margin loss) on 8
Trainium2 NeuronCores, pure data parallel.

Math (reference):
    norms = ||x_i||;  soft = relu(1.5 - r) + relu(r - 2)   (r = norms)
    xn = x / max(r, eps);  wf = xn @ W.T   (W is [10, 2])
    t = wf[i, label_i];  num = S*cos(arccos(clip(t)) + M)
    den = exp(num) + sum_c exp(S*wf_c) - exp(S*t)
    loss = -mean(num - log(den)) + LBDA*mean(soft)/2

Kernel strategy (per core, 524288 rows as [128 partitions x 4096], two
passes of F=2048):
  - ScalarE stays in the single natural_log_exp table set: all sqrt/rsqrt
    are computed as Exp(k*Ln(.)); cos(arccos t + M) = cosM*t - S*sinM*sqrt(u)
    with u = 1-(t)^2, so no trig tables are needed.
  - Per-class dots z_c = S/r * (x0*w0c + x1*w1c) are built on VectorE in
    bf16 via tensor_scalar with per-partition weight scalars.
  - sum_c exp(z_c) and the label-selected exp(z_l) are accumulated on the
    otherwise-idle TensorE: identity-matmul PSUM accumulation over the 10
    class tiles (e_c and mask_c*e_c).  tgt = Ln(e_l).
  - Per-row loss terms are reduced on-chip with fused accum_out reductions
    into a [128, 8] partials tile; the host sums 8 cores x [128, 8].
"""

import math
import os
import sys

import numpy as np

for _p in ("/opt/trn_rl_repo", "/root/.axon_site/_ro/trn_rl_repo"):
    if os.path.isdir(_p) and _p not in sys.path:
        sys.path.insert(0, _p)

from contextlib import ExitStack

from concourse import bacc, bass, tile
from concourse import mybir
from concourse.bass_utils import run_bass_kernel_spmd

# ---- problem constants (hardcoded; kernel.py must be self-contained) ----
S = 30.0
M = 0.5
LBDA = 1.0
EPS = 1e-7
N = 4_194_304
N_CORES = 8
P = 128
NC_ROWS = N // N_CORES            # 524288 rows per core
PF = NC_ROWS // P                 # 4096 per partition
F = 1024                          # free-dim per pass
NPASS = PF // F                   # 2
NCLS = 10
MM_N = 512                        # one PSUM bank of fp32 per matmul

COS_M = math.cos(M)
TAN_M = math.tan(M)
CLIP_HI = S * (1.0 - EPS)
CLIP_LO = -S * (1.0 - EPS)

f32 = mybir.dt.float32
f32r = mybir.dt.float32r
bf16 = mybir.dt.bfloat16
i32 = mybir.dt.int32
Alu = mybir.AluOpType
Act = mybir.ActivationFunctionType

# staged build for hardware debugging: 4 = full kernel
K_STAGE = int(os.environ.get("K_STAGE", "4"))


_CONST_BIASES = (1e-30, 1.5, -2.0, math.log(S), math.log(S * TAN_M), 1e-12)


def _patch_act_tables():
    """Force all our activation functions onto the one table set that
    contains them all (natural_log_exp_and_others).  The default greedy
    chooser puts ln and exp in different sets, paying a ~2.7us table
    reload at every ln<->exp boundary (29 loads = ~78us per kernel)."""
    import concourse.hw_specs as hw_specs
    import concourse.bacc as bacc_mod

    orig = hw_specs.get_activation_tables
    if getattr(bacc_mod.get_activation_tables, "_k_patched", False):
        return
    ours = {Act.Exp, Act.Ln, Act.Square, Act.Relu, Act.Copy, Act.Identity}

    def patched(module_arch):
        tables = orig(module_arch)
        target = "natural_log_exp_and_others"
        assert target in tables and ours <= tables[target], (
            target, tables.get(target))
        for name in tables:
            if name != target:
                tables[name] = tables[name] - ours
        return tables

    patched._k_patched = True
    bacc_mod.get_activation_tables = patched


def _build_graph():
    _patch_act_tables()
    nc = bacc.Bacc(
        "TRN2", target_bir_lowering=False, debug=False, enable_asserts=False
    )
    for i, v in enumerate(_CONST_BIASES):
        t = nc.alloc_sbuf_tensor(f"kconst-{i}", [P, 1], f32)
        nc.gpsimd.memset(t.ap(), v)
        nc.const_aps.aps[(f32, v)] = t.ap()
    nc.all_engine_barrier()
    x0_d = nc.dram_tensor("x0", [P, PF], f32, kind="ExternalInput").ap()
    x1_d = nc.dram_tensor("x1", [P, PF], f32, kind="ExternalInput").ap()
    lbl_d = nc.dram_tensor("lbl", [P, PF], bf16, kind="ExternalInput").ap()
    wq_d = nc.dram_tensor("wq", [P, 2 * NCLS], f32, kind="ExternalInput").ap()
    id_d = nc.dram_tensor("ident", [P, P], f32, kind="ExternalInput").ap()
    out_d = nc.dram_tensor("out", [P, 4 * NPASS], f32, kind="ExternalOutput").ap()
    dbg_d = None
    if os.environ.get("K_DEBUG", "0") == "1":
        dbg_d = [
            nc.dram_tensor(f"dbg{i}", [P, F], f32, kind="ExternalOutput").ap()
            for i in range(12)
        ]

    with tile.TileContext(nc) as tc, ExitStack() as ctx:
        _emit(ctx, tc, nc, x0_d, x1_d, lbl_d, wq_d, id_d, out_d, dbg_d)
    nc.compile()
    return nc


def _emit(ctx, tc, nc, x0_d, x1_d, lbl_d, wq_d, id_d, out_d, dbg_d=None):
    const = ctx.enter_context(tc.tile_pool(name="const", bufs=1))
    dma_p = ctx.enter_context(tc.tile_pool(name="dma", bufs=3))
    f32s = ctx.enter_context(tc.tile_pool(name="f32s", bufs=1))
    bfs = ctx.enter_context(tc.tile_pool(name="bfs", bufs=2))
    rot = ctx.enter_context(tc.tile_pool(name="rot", bufs=5))
    psum = ctx.enter_context(tc.tile_pool(name="psum", bufs=2, space="PSUM"))

    # one-time constants
    wq = const.tile([P, 2 * NCLS], f32, tag="wq")
    nc.sync.dma_start(wq[:], wq_d[:])
    idf = const.tile([P, P], f32r, tag="idf")
    nc.sync.dma_start(idf[:], id_d[:].bitcast(f32r))
    idn = const.tile([P, P], f32, tag="idn")
    nc.vector.tensor_scalar(idn[:], idf[:], -1.0, None, Alu.mult)
    sacc = const.tile([P, 4 * NPASS], f32, tag="sacc")

    repeat = int(os.environ.get("K_REPEAT", "0"))
    if repeat > 1:
        ctx.enter_context(tc.For_i(0, repeat, 1))

    npass = 1 if (dbg_d is not None and os.environ.get("K_ONEPASS", "0") == "1") else NPASS

    deferred = []

    def head_and_classes(t):
        sl = bass.ts(t, F)

        x0t = dma_p.tile([P, F], f32, tag="x0t")
        nc.sync.dma_start(x0t[:], x0_d[:, sl])
        x1t = dma_p.tile([P, F], f32, tag="x1t")
        nc.sync.dma_start(x1t[:], x1_d[:, sl])
        lblt = dma_p.tile([P, F], bf16, tag="lblt")
        nc.sync.dma_start(lblt[:], lbl_d[:, sl])

        # ---- per-row scalars: r and S/r, via Ln/Exp only ----
        sq0 = f32s.tile([P, F], f32, tag="sq0")
        nc.vector.tensor_mul(sq0[:], x0t[:], x0t[:])
        sq1 = f32s.tile([P, F], f32, tag="sq1")
        nc.vector.tensor_mul(sq1[:], x1t[:], x1t[:])
        rsq = f32s.tile([P, F], f32, tag="rsq")
        nc.vector.tensor_add(rsq[:], sq0[:], sq1[:])

        lr = f32s.tile([P, F], f32, tag="lr")
        nc.scalar.activation(lr[:], rsq[:], Act.Ln, bias=1e-30)
        r = f32s.tile([P, F], f32, tag="r")
        nc.scalar.activation(r[:], lr[:], Act.Exp, scale=0.5)
        sinvr = f32s.tile([P, F], f32, tag="sinvr")
        nc.scalar.activation(sinvr[:], lr[:], Act.Exp, bias=math.log(S), scale=-0.5)

        # ---- soft loss: relu(1.5 - r) + relu(r - 2), summed ----
        trash = f32s.tile([P, F], f32, tag="trash")
        nc.scalar.activation(
            trash[:], r[:], Act.Relu, bias=1.5, scale=-1.0,
            accum_out=sacc[:, 4 * t + 2 : 4 * t + 3],
        )
        trash_b = f32s.tile([P, F], f32, tag="trash")
        nc.scalar.activation(
            trash_b[:], r[:], Act.Relu, bias=-2.0, scale=1.0,
            accum_out=sacc[:, 4 * t + 3 : 4 * t + 4],
        )

        # ---- scaled unit vectors in bf16: y = (S/r) * x ----
        y0b = bfs.tile([P, F], bf16, tag="y0b")
        nc.vector.tensor_mul(y0b[:], x0t[:], sinvr[:])
        y1b = bfs.tile([P, F], bf16, tag="y1b")
        nc.vector.tensor_mul(y1b[:], x1t[:], sinvr[:])

        # ---- per-class: z_c (bf16), e_c = exp(z_c) (f32), mask_c * e_c ----
        # fp32 identity-matmul accumulation on TensorE (bf16 matmul is
        # sparsely corrupt on this stack; fp32 verified exact)
        es_ps = psum.tile([P, F], f32, tag="es")
        el_ps = psum.tile([P, F], f32, tag="el")
        for c in range(NCLS):
            za = rot.tile([P, F], bf16, tag="za")
            nc.vector.tensor_scalar(
                za[:], y0b[:], wq[:, 2 * c : 2 * c + 1], None, Alu.mult
            )
            zb = rot.tile([P, F], bf16, tag="zb")
            nc.vector.tensor_scalar(
                zb[:], y1b[:], wq[:, 2 * c + 1 : 2 * c + 2], None, Alu.mult
            )
            zc = rot.tile([P, F], bf16, tag="zc")
            nc.vector.tensor_add(zc[:], za[:], zb[:])

            ecb = rot.tile([P, F], f32r, tag="ecb")
            nc.scalar.activation(ecb[:], zc[:], Act.Exp)

            mcb = rot.tile([P, F], bf16, tag="mcb")
            nc.vector.tensor_scalar(mcb[:], lblt[:], float(c), None, Alu.is_equal)
            mz = rot.tile([P, F], f32r, tag="mz")
            nc.vector.tensor_mul(mz[:], mcb[:], ecb[:])

            for k in range(F // MM_N):
                ck = bass.ts(k, MM_N)
                nc.tensor.matmul(
                    es_ps[:, ck], idf[:], ecb[:, ck],
                    start=(c == 0), stop=(c == NCLS - 1),
                )
                nc.tensor.matmul(
                    el_ps[:, ck], idf[:], mz[:, ck],
                    start=(c == 0), stop=(c == NCLS - 1),
                )

        return es_ps, el_ps

    def tail(t, es_ps, el_ps):
        # ---- target logit: tgt_S = Ln(e_l); numerator ----
        tgts = f32s.tile([P, F], f32, tag="tgts")
        nc.scalar.activation(tgts[:], el_ps[:], Act.Ln)
        tcl = f32s.tile([P, F], f32, tag="tcl")
        nc.vector.tensor_scalar(tcl[:], tgts[:], CLIP_HI, CLIP_LO, Alu.min, Alu.max)
        t2 = f32s.tile([P, F], f32, tag="t2")
        nc.scalar.activation(t2[:], tcl[:], Act.Square, scale=1.0 / S)
        u = f32s.tile([P, F], f32, tag="u")
        nc.vector.tensor_scalar(u[:], t2[:], -1.0, 1.0, Alu.mult, Alu.add)
        lnu = f32s.tile([P, F], f32, tag="lnu")
        nc.scalar.activation(lnu[:], u[:], Act.Ln, bias=1e-12)
        sqru = f32s.tile([P, F], f32, tag="sqru")
        nc.scalar.activation(
            sqru[:], lnu[:], Act.Exp, bias=math.log(S * TAN_M), scale=0.5
        )
        # num = (tcl - S*tanM*sqrt(u)) * cosM ; Copy-activation accumulates sum
        nump = f32s.tile([P, F], f32, tag="nump")
        nc.vector.tensor_tensor(nump[:], tcl[:], sqru[:], Alu.subtract)
        numt = f32s.tile([P, F], f32, tag="numt")
        nc.scalar.activation(
            numt[:], nump[:], Act.Copy, scale=COS_M,
            accum_out=sacc[:, 4 * t + 0 : 4 * t + 1],
        )

        # ---- denominator & log ----
        e_num = f32s.tile([P, F], f32, tag="e_num")
        nc.scalar.activation(e_num[:], numt[:], Act.Exp)
        d1 = f32s.tile([P, F], f32, tag="d1")
        nc.vector.tensor_add(d1[:], e_num[:], es_ps[:])
        den = f32s.tile([P, F], f32, tag="den")
        nc.vector.tensor_tensor(den[:], d1[:], el_ps[:], Alu.subtract)
        trash2 = f32s.tile([P, F], f32, tag="trash")
        nc.scalar.activation(
            trash2[:], den[:], Act.Ln,
            accum_out=sacc[:, 4 * t + 1 : 4 * t + 2],
        )
        if dbg_d is not None and t == 0:
            def dump(i, src_ap):
                dtile = f32s.tile([P, F], f32, tag=f"dmp{i}", name=f"dmp{i}")
                nc.vector.tensor_copy(dtile[:], src_ap)
                nc.sync.dma_start(dbg_d[i][:], dtile[:])
            dump(0, x0t[:])
            dump(1, sinvr[:])
            dump(2, y0b[:])
            dump(3, y1b[:])
            dump(4, lblb[:])
            dump(5, es_ps[:])
            dump(6, el_ps[:])
            dump(7, tgts[:])
            dump(8, tcl[:])
            dump(9, sqru[:])
            dump(10, numt[:])
            dump(11, trash2[:])

    for t in range(npass):
        ps = head_and_classes(t)
        deferred.append((t, ps))
        if len(deferred) > 1:
            tp, (es, el) = deferred.pop(0)
            tail(tp, es, el)
    for tp, (es, el) in deferred:
        tail(tp, es, el)

    nc.sync.dma_start(out_d[:], sacc[:])


_NC_CACHE = None


def _get_graph():
    global _NC_CACHE
    if _NC_CACHE is None:
        _NC_CACHE = _build_graph()
    return _NC_CACHE


def kernel(x, labels, weight):
    x = np.asarray(x, dtype=np.float32)
    import ml_dtypes
    labels = np.asarray(labels).astype(ml_dtypes.bfloat16)
    w = np.asarray(weight, dtype=np.float32)

    nc = _get_graph()

    wq = np.ascontiguousarray(np.tile(w.reshape(1, 2 * NCLS), (P, 1)))
    ident = np.eye(P, dtype=np.float32)

    in_maps = []
    for i in range(N_CORES):
        xs = x[i * NC_ROWS : (i + 1) * NC_ROWS]
        ls = labels[i * NC_ROWS : (i + 1) * NC_ROWS]
        in_maps.append(
            {
                "x0": np.ascontiguousarray(xs[:, 0]).reshape(P, PF),
                "x1": np.ascontiguousarray(xs[:, 1]).reshape(P, PF),
                "lbl": np.ascontiguousarray(ls).reshape(P, PF),
                "wq": wq,
                "ident": ident,
            }
        )

    trace = os.environ.get("KTRACE", "0") == "1"
    res = run_bass_kernel_spmd(nc, in_maps, core_ids=list(range(N_CORES)), trace=trace)
    if getattr(res, "exec_time_ns", None):
        print(f"HW exec time: {res.exec_time_ns} ns")

    num_sum = 0.0
    lden_sum = 0.0
    soft_sum = 0.0
    for i in range(N_CORES):
        o = np.asarray(res.results[i]["out"], dtype=np.float64)
        for t in range(NPASS):
            num_sum += o[:, 4 * t + 0].sum()
            lden_sum += o[:, 4 * t + 1].sum()
            soft_sum += o[:, 4 * t + 2].sum() + o[:, 4 * t + 3].sum()

    loss = -(num_sum - lden_sum) / N + LBDA * (soft_sum / N) / 2.0
    return np.float32(loss)


if __name__ == "__main__":
    # smoke test with random data
    rng = np.random.default_rng(0)
    x = rng.standard_normal((N, 2), dtype=np.float32)
    labels = rng.integers(0, 10, size=(N,)).astype(np.int64)
    w = np.array(
        [[1, 0], [0.809, 0.588], [0.309, 0.951], [-0.309, 0.951], [-0.809, 0.588],
         [-1, 0], [-0.809, -0.588], [-0.309, -0.951], [0.309, -0.951], [0.809, -0.588]],
        dtype=np.float32,
    )
    print(kernel(x, labels, w))



# revision 10
# speedup vs baseline: 1.5871x; 1.5871x over previous
"""
AngularPenaltySMLoss on 8 Trainium2 NeuronCores, pure data parallel.

Math (reference):
    r = ||x_i||;  soft = relu(1.5 - r) + relu(r - 2)
    xn = x / max(r, eps);  wf = xn @ W.T   (W is [10, 2])
    t = wf[i, label_i];  num = S*cos(arccos(clip(t)) + M)
    den = exp(num) + sum_c exp(S*wf_c) - exp(S*t)
    loss = -mean(num - log(den)) + LBDA*mean(soft)/2

Kernel strategy (per core, 524288 rows as [128 x 4096], two passes of
F=2048, all VectorE elementwise in bf16):
  - Host pre-gathers the target-class weights wl = W_bf16[label] and ships
    them as two bf16 planes: the target dot tt = y0*wl0 + y1*wl1 is then
    BIT-IDENTICAL to the class dot z_label (same inputs, same ops), so
    excl_sum = sum_c exp(S z_c) - exp(S tt) cancels exactly -- no
    per-class label masks needed at all.
  - W has w[c+5] == -w[c] exactly (checked on host): only 5 class dots are
    computed; exp(+S z) and exp(-S z) come from two ScalarE activations on
    the same tile (scale=+-S). Falls back to a 10-dot graph otherwise.
  - sum_c exp(.) - exp(S tt) accumulates on TensorE via identity-matmul
    chains into PSUM (idm = -I for the target term).
  - Whole tail folds into one accumulated Ln:
      -L = ln(1 + excl * e^{-num}) summed by the activation's accum_out.
    num is never materialized or summed; exp(-num) comes from one Exp.
  - softloss partial sums ride free on tensor_scalar accum_out:
      a = min(r,1.5)-1.5 (= -relu(1.5-r)),  b = max(r,2)-2 (= relu(r-2)).
  - Host sums the 8 cores' [128, 6] partial tiles.
"""

import math
import os
import sys

import numpy as np

for _p in ("/opt/trn_rl_repo", "/root/.axon_site/_ro/trn_rl_repo"):
    if os.path.isdir(_p) and _p not in sys.path:
        sys.path.insert(0, _p)

from contextlib import ExitStack

from concourse import bacc, bass, tile
from concourse import mybir
from concourse.bass_utils import run_bass_kernel_spmd

# ---- problem constants (hardcoded; kernel.py must be self-contained) ----
S = 30.0
M = 0.5
LBDA = 1.0
N = 4_194_304
N_CORES = 8
P = 128
NC_ROWS = N // N_CORES            # 524288 rows per core
PF = NC_ROWS // P                 # 4096 per partition
F = 2048                          # free-dim per pass
NPASS = PF // F                   # 2
NCLS = 10
MM_N = 512                        # one PSUM bank of fp32 per matmul

COS_M = math.cos(M)
TAN_M = math.tan(M)
LN_TAN_M = math.log(TAN_M)

f32 = mybir.dt.float32
f32r = mybir.dt.float32r
bf16 = mybir.dt.bfloat16
Alu = mybir.AluOpType
Act = mybir.ActivationFunctionType

# ln(1 + excl*e^-num) can reach e^58, past ScalarE Ln's 2^64 domain: shift
# everything by e^-LSH (folded into e_nn's bias and the Ln's bias; host
# adds LSH back per row).
LSH = 30.0
_CONST_BIASES = (1e-30, 1e-12, LN_TAN_M, math.exp(-LSH), -LSH)


def _patch_act_tables():
    """Force Exp and Ln onto the one table set containing both
    (natural_log_exp_and_others) so no ~2.7us table reloads occur at
    ln<->exp boundaries."""
    import concourse.hw_specs as hw_specs
    import concourse.bacc as bacc_mod

    orig = hw_specs.get_activation_tables
    if getattr(bacc_mod.get_activation_tables, "_k_patched", False):
        return
    ours = {Act.Exp, Act.Ln, Act.Copy, Act.Identity}

    def patched(module_arch):
        tables = orig(module_arch)
        target = "natural_log_exp_and_others"
        assert target in tables and ours <= tables[target], (
            target, tables.get(target))
        for name in tables:
            if name != target:
                tables[name] = tables[name] - ours
        return tables

    patched._k_patched = True
    bacc_mod.get_activation_tables = patched


def _build_graph(sym: bool):
    _patch_act_tables()
    nc = bacc.Bacc(
        "TRN2", target_bir_lowering=False, debug=False, enable_asserts=False
    )
    for i, v in enumerate(_CONST_BIASES):
        t = nc.alloc_sbuf_tensor(f"kconst-{i}", [P, 1], f32)
        nc.gpsimd.memset(t.ap(), v)
        nc.const_aps.aps[(f32, v)] = t.ap()
    nc.all_engine_barrier()
    npairs = 5 if sym else NCLS
    x0_d = nc.dram_tensor("x0", [P, PF], bf16, kind="ExternalInput").ap()
    x1_d = nc.dram_tensor("x1", [P, PF], bf16, kind="ExternalInput").ap()
    wl0_d = nc.dram_tensor("wl0", [P, PF], bf16, kind="ExternalInput").ap()
    wl1_d = nc.dram_tensor("wl1", [P, PF], bf16, kind="ExternalInput").ap()
    wq_d = nc.dram_tensor("wq", [P, 2 * npairs], f32, kind="ExternalInput").ap()
    idp_d = nc.dram_tensor("idp", [P, P], f32, kind="ExternalInput").ap()
    idm_d = nc.dram_tensor("idm", [P, P], f32, kind="ExternalInput").ap()
    out_d = nc.dram_tensor("out", [P, 3 * NPASS], f32, kind="ExternalOutput").ap()

    with tile.TileContext(nc) as tc, ExitStack() as ctx:
        _emit(ctx, tc, nc, sym, x0_d, x1_d, wl0_d, wl1_d, wq_d, idp_d, idm_d, out_d)
    nc.compile()
    return nc


def _emit(ctx, tc, nc, sym, x0_d, x1_d, wl0_d, wl1_d, wq_d, idp_d, idm_d, out_d):
    npairs = 5 if sym else NCLS
    const = ctx.enter_context(tc.tile_pool(name="const", bufs=1))
    dma_p = ctx.enter_context(tc.tile_pool(name="dma", bufs=2))
    h1 = ctx.enter_context(tc.tile_pool(name="h1", bufs=1))
    h2 = ctx.enter_context(tc.tile_pool(name="h2", bufs=2))
    zab = ctx.enter_context(tc.tile_pool(name="zab", bufs=1))
    zcp = ctx.enter_context(tc.tile_pool(name="zcp", bufs=2))
    ep = ctx.enter_context(tc.tile_pool(name="ep", bufs=3))
    etp = ctx.enter_context(tc.tile_pool(name="etp", bufs=2))
    tl = ctx.enter_context(tc.tile_pool(name="tl", bufs=1))
    psum = ctx.enter_context(tc.tile_pool(name="psum", bufs=2, space="PSUM"))

    # one-time constants
    wq = const.tile([P, 2 * npairs], f32, tag="wq")
    nc.sync.dma_start(wq[:], wq_d[:])
    idp = const.tile([P, P], f32r, tag="idp")
    nc.sync.dma_start(idp[:], idp_d[:].bitcast(f32r))
    idm = const.tile([P, P], f32r, tag="idm")
    nc.sync.dma_start(idm[:], idm_d[:].bitcast(f32r))
    sacc = const.tile([P, 3 * NPASS], f32, tag="sacc")

    state = {}

    def head(t):
        sl = bass.ts(t, F)
        x0t = dma_p.tile([P, F], bf16, tag="x0t")
        nc.sync.dma_start(x0t[:], x0_d[:, sl])
        x1t = dma_p.tile([P, F], bf16, tag="x1t")
        nc.sync.dma_start(x1t[:], x1_d[:, sl])
        wl0t = dma_p.tile([P, F], bf16, tag="wl0t")
        nc.sync.dma_start(wl0t[:], wl0_d[:, sl])
        wl1t = dma_p.tile([P, F], bf16, tag="wl1t")
        nc.sync.dma_start(wl1t[:], wl1_d[:, sl])

        # r^2 and 1/r (via Ln/Exp)
        sq0 = h1.tile([P, F], bf16, tag="sq0")
        nc.vector.tensor_mul(sq0[:], x0t[:], x0t[:])
        sq1 = h1.tile([P, F], bf16, tag="sq1")
        nc.vector.tensor_mul(sq1[:], x1t[:], x1t[:])
        rsq = h1.tile([P, F], bf16, tag="rsq")
        nc.vector.tensor_add(rsq[:], sq0[:], sq1[:])
        lr = h1.tile([P, F], f32, tag="lr")
        nc.scalar.activation(lr[:], rsq[:], Act.Ln, bias=1e-30)
        invr = h1.tile([P, F], bf16, tag="invr")
        nc.scalar.activation(invr[:], lr[:], Act.Exp, scale=-0.5)

        # unit vector, r, soft partial sums (ride on tensor_scalar accum)
        y0 = h2.tile([P, F], bf16, tag="y0")
        nc.vector.tensor_mul(y0[:], x0t[:], invr[:])
        y1 = h2.tile([P, F], bf16, tag="y1")
        nc.vector.tensor_mul(y1[:], x1t[:], invr[:])
        rs = h1.tile([P, F], bf16, tag="rs")
        nc.vector.tensor_mul(rs[:], rsq[:], invr[:])
        # accum_out = reduce<op1>(op0 result): Sa = sum(min(r,1.5)),
        # Sb = sum(max(r,2)); soft folds on host as Sb - Sa - 0.5*N.
        trash_a = h1.tile([P, F], bf16, tag="trash_a")
        nc.vector.tensor_scalar(
            trash_a[:], rs[:], 1.5, None, Alu.min, Alu.add,
            accum_out=sacc[:, 3 * t + 1 : 3 * t + 2],
        )
        trash_b = h1.tile([P, F], bf16, tag="trash_b")
        nc.vector.tensor_scalar(
            trash_b[:], rs[:], 2.0, None, Alu.max, Alu.add,
            accum_out=sacc[:, 3 * t + 2 : 3 * t + 3],
        )

        # target dot: bit-identical to the class-c dot for c == label
        t1 = h1.tile([P, F], bf16, tag="t1")
        nc.vector.tensor_mul(t1[:], y0[:], wl0t[:])
        t2 = h1.tile([P, F], bf16, tag="t2")
        nc.vector.tensor_mul(t2[:], y1[:], wl1t[:])
        tt = h2.tile([P, F], bf16, tag="tt")
        nc.vector.tensor_add(tt[:], t1[:], t2[:])

        # tail-pre (only needs tt; keeps ScalarE fed later)
        tcl = tl.tile([P, F], bf16, tag="tcl")
        nc.vector.tensor_scalar(tcl[:], tt[:], 1.0, -1.0, Alu.min, Alu.max)
        tsq = tl.tile([P, F], bf16, tag="tsq")
        nc.vector.tensor_mul(tsq[:], tcl[:], tcl[:])
        u = tl.tile([P, F], bf16, tag="u")
        nc.vector.tensor_scalar(u[:], tsq[:], -1.0, 1.0, Alu.mult, Alu.add)

        e_t = etp.tile([P, F], f32r, tag="et")
        nc.scalar.activation(e_t[:], tt[:], Act.Exp, scale=S)

        es_ps = psum.tile([P, F], f32, tag="es")
        for k in range(F // MM_N):
            ck = bass.ts(k, MM_N)
            nc.tensor.matmul(es_ps[:, ck], idm[:], e_t[:, ck], start=True, stop=False)

        state[t] = dict(y0=y0, y1=y1, tt=tt, tcl=tcl, u=u, es=es_ps)

    def classes(t, spliced):
        """Class dot/exp/matmul loop for pass t; `spliced` maps a class
        index to a list of closures emitted after that class (the previous
        pass's tail, interleaved to hide V<->S ping-pong)."""
        st = state[t]
        y0, y1, es_ps = st["y0"], st["y1"], st["es"]
        n_exp = 2 if sym else 1
        for c in range(npairs):
            za = zab.tile([P, F], bf16, tag="za")
            nc.vector.tensor_scalar(
                za[:], y0[:], wq[:, 2 * c : 2 * c + 1], None, Alu.mult
            )
            zb = zab.tile([P, F], bf16, tag="zb")
            nc.vector.tensor_scalar(
                zb[:], y1[:], wq[:, 2 * c + 1 : 2 * c + 2], None, Alu.mult
            )
            zc = zcp.tile([P, F], bf16, tag="zc")
            nc.vector.tensor_add(zc[:], za[:], zb[:])
            for j in range(n_exp):
                e = ep.tile([P, F], f32r, tag="e")
                nc.scalar.activation(e[:], zc[:], Act.Exp, scale=(S, -S)[j])
                last = (c == npairs - 1) and (j == n_exp - 1)
                for k in range(F // MM_N):
                    ck = bass.ts(k, MM_N)
                    nc.tensor.matmul(
                        es_ps[:, ck], idp[:], e[:, ck], start=False, stop=last
                    )
            for fn in spliced.get(c, ()):
                fn()

    def tail_post(t):
        """Everything after pass t's denominator sum is ready."""
        st = state[t]

        def s_lnu_sqru():
            lnu = tl.tile([P, F], bf16, tag="lnu")
            nc.scalar.activation(lnu[:], st["u"][:], Act.Ln, bias=1e-12)
            sqru = tl.tile([P, F], bf16, tag="sqru")
            nc.scalar.activation(sqru[:], lnu[:], Act.Exp, scale=0.5, bias=LN_TAN_M)
            st["sqru"] = sqru

        def v_nump():
            nump = tl.tile([P, F], bf16, tag="nump")
            nc.vector.tensor_tensor(nump[:], st["tcl"][:], st["sqru"][:], Alu.subtract)
            st["nump"] = nump

        def s_enn():
            e_nn = tl.tile([P, F], f32, tag="enn")
            nc.scalar.activation(
                e_nn[:], st["nump"][:], Act.Exp, scale=-S * COS_M, bias=-LSH
            )
            st["enn"] = e_nn

        def v_w():
            w = tl.tile([P, F], f32, tag="w")
            nc.vector.tensor_mul(w[:], st["es"][:], st["enn"][:])
            st["w"] = w

        def s_lnacc():
            trash = tl.tile([P, F], bf16, tag="trash_l")
            nc.scalar.activation(
                trash[:], st["w"][:], Act.Ln, bias=math.exp(-LSH),
                accum_out=sacc[:, 3 * t : 3 * t + 1],
            )
            del state[t]

        return [s_lnu_sqru, v_nump, s_enn, v_w, s_lnacc]

    mid = max(1, npairs // 2 - 1)
    for t in range(NPASS):
        head(t)
        spliced = {}
        if t > 0:
            ops = tail_post(t - 1)
            # spread the 5 tail closures across the class loop
            for i, fn in enumerate(ops):
                spliced.setdefault(min(mid + i, npairs - 1), []).append(fn)
        classes(t, spliced)
    for fn in tail_post(NPASS - 1):
        fn()

    nc.sync.dma_start(out_d[:], sacc[:])


_NC_CACHE = {}


def _get_graph(sym: bool):
    if sym not in _NC_CACHE:
        _NC_CACHE[sym] = _build_graph(sym)
    return _NC_CACHE[sym]


def kernel(x, labels, weight):
    import ml_dtypes

    BF = ml_dtypes.bfloat16
    x = np.asarray(x, dtype=np.float32)
    labels = np.asarray(labels).astype(np.int64)
    w = np.asarray(weight, dtype=np.float32)

    wb = w.astype(BF)
    sym = w.shape[0] == NCLS and np.array_equal(
        wb[NCLS // 2 :], -wb[: NCLS // 2]
    )
    npairs = 5 if sym else NCLS
    nc = _get_graph(sym)

    wq = np.ascontiguousarray(
        np.tile(wb[:npairs].astype(np.float32).reshape(1, 2 * npairs), (P, 1))
    )
    idp = np.eye(P, dtype=np.float32)
    idm = -idp

    wl = wb[labels]                      # [N, 2] bf16 gather on host
    x0 = x[:, 0].astype(BF)
    x1 = x[:, 1].astype(BF)

    in_maps = []
    for i in range(N_CORES):
        sl = slice(i * NC_ROWS, (i + 1) * NC_ROWS)
        in_maps.append(
            {
                "x0": np.ascontiguousarray(x0[sl]).reshape(P, PF),
                "x1": np.ascontiguousarray(x1[sl]).reshape(P, PF),
                "wl0": np.ascontiguousarray(wl[sl, 0]).reshape(P, PF),
                "wl1": np.ascontiguousarray(wl[sl, 1]).reshape(P, PF),
                "wq": wq,
                "idp": idp,
                "idm": idm,
            }
        )

    trace = os.environ.get("KTRACE", "0") == "1"
    res = run_bass_kernel_spmd(nc, in_maps, core_ids=list(range(N_CORES)), trace=trace)
    if getattr(res, "exec_time_ns", None):
        print(f"HW exec time: {res.exec_time_ns} ns")

    lden_sum = 0.0
    soft_sum = -0.5 * N
    for i in range(N_CORES):
        o = np.asarray(res.results[i]["out"], dtype=np.float64)
        for t in range(NPASS):
            lden_sum += o[:, 3 * t + 0].sum()
            soft_sum += o[:, 3 * t + 2].sum() - o[:, 3 * t + 1].sum()

    loss = lden_sum / N + LSH + (LBDA / 2.0) * (soft_sum / N)
    return np.float32(loss)


if __name__ == "__main__":
    rng = np.random.default_rng(0)
    x = rng.standard_normal((N, 2), dtype=np.float32)
    labels = rng.integers(0, 10, size=(N,)).astype(np.int64)
    w = np.array(
        [[1, 0], [0.809, 0.588], [0.309, 0.951], [-0.309, 0.951], [-0.809, 0.588],
         [-1, 0], [-0.809, -0.588], [-0.309, -0.951], [0.309, -0.951], [0.809, -0.588]],
        dtype=np.float32,
    )
    print(kernel(x, labels, w))


# revision 21
# speedup vs baseline: 1.7784x; 1.1205x over previous
"""
AngularPenaltySMLoss on 8 Trainium2 NeuronCores, pure data parallel.

Math (reference):
    r = ||x_i||;  soft = relu(1.5 - r) + relu(r - 2)
    xn = x / max(r, eps);  wf = xn @ W.T   (W is [10, 2])
    t = wf[i, label_i];  num = S*cos(arccos(clip(t)) + M)
    den = exp(num) + sum_c exp(S*wf_c) - exp(S*t)
    loss = -mean(num - log(den)) + LBDA*mean(soft)/2

Kernel strategy (per core, 524288 rows as [128 x 4096], two passes of
F=2048, all VectorE elementwise in bf16):
  - Host pre-gathers the target-class weights wl = W_bf16[label] and ships
    them as two bf16 planes: the target dot tt = y0*wl0 + y1*wl1 is then
    BIT-IDENTICAL to the class dot z_label (same inputs, same ops), so
    excl_sum = sum_c exp(S z_c) - exp(S tt) cancels exactly -- no
    per-class label masks needed at all.
  - W has w[c+5] == -w[c] exactly (checked on host): only 5 class dots are
    computed; exp(+S z) and exp(-S z) come from two ScalarE activations on
    the same tile (scale=+-S). Falls back to a 10-dot graph otherwise.
  - sum_c exp(.) - exp(S tt) accumulates on TensorE via identity-matmul
    chains into PSUM (idm = -I for the target term).
  - Whole tail folds into one accumulated Ln:
      -L = ln(1 + excl * e^{-num}) summed by the activation's accum_out.
    num is never materialized or summed; exp(-num) comes from one Exp.
  - softloss partial sums ride free on tensor_scalar accum_out:
      a = min(r,1.5)-1.5 (= -relu(1.5-r)),  b = max(r,2)-2 (= relu(r-2)).
  - Host sums the 8 cores' [128, 6] partial tiles.
"""

import math
import os
import sys

import numpy as np

for _p in ("/opt/trn_rl_repo", "/root/.axon_site/_ro/trn_rl_repo"):
    if os.path.isdir(_p) and _p not in sys.path:
        sys.path.insert(0, _p)

from contextlib import ExitStack

from concourse import bacc, bass, tile
from concourse import mybir
from concourse.bass_utils import run_bass_kernel_spmd

# ---- problem constants (hardcoded; kernel.py must be self-contained) ----
S = 30.0
M = 0.5
LBDA = 1.0
N = 4_194_304
N_CORES = 8
P = 128
NC_ROWS = N // N_CORES            # 524288 rows per core
PF = NC_ROWS // P                 # 4096 per partition
F = 2048                          # free-dim per pass
NPASS = PF // F                   # 2
NCLS = 10
MM_N = 512                        # one PSUM bank of fp32 per matmul

COS_M = math.cos(M)
TAN_M = math.tan(M)
LN_TAN_M = math.log(TAN_M)

f32 = mybir.dt.float32
f32r = mybir.dt.float32r
bf16 = mybir.dt.bfloat16
Alu = mybir.AluOpType
Act = mybir.ActivationFunctionType

# ln(1 + excl*e^-num) can reach e^58, past ScalarE Ln's 2^64 domain: shift
# everything by e^-LSH (folded into e_nn's bias and the Ln's bias; host
# adds LSH back per row).
LSH = 30.0
_CONST_BIASES = (1e-30, 1e-12, LN_TAN_M, math.exp(-LSH), -LSH)


def _patch_act_tables():
    """Force Exp and Ln onto the one table set containing both
    (natural_log_exp_and_others) so no ~2.7us table reloads occur at
    ln<->exp boundaries."""
    import concourse.hw_specs as hw_specs
    import concourse.bacc as bacc_mod

    orig = hw_specs.get_activation_tables
    if getattr(bacc_mod.get_activation_tables, "_k_patched", False):
        return
    ours = {Act.Exp, Act.Ln, Act.Copy, Act.Identity}

    def patched(module_arch):
        tables = orig(module_arch)
        target = "natural_log_exp_and_others"
        assert target in tables and ours <= tables[target], (
            target, tables.get(target))
        for name in tables:
            if name != target:
                tables[name] = tables[name] - ours
        return tables

    patched._k_patched = True
    bacc_mod.get_activation_tables = patched


def _build_graph(sym: bool):
    _patch_act_tables()
    nc = bacc.Bacc(
        "TRN2", target_bir_lowering=False, debug=False, enable_asserts=False
    )
    for i, v in enumerate(_CONST_BIASES):
        t = nc.alloc_sbuf_tensor(f"kconst-{i}", [P, 1], f32)
        nc.gpsimd.memset(t.ap(), v)
        nc.const_aps.aps[(f32, v)] = t.ap()
    nc.all_engine_barrier()
    npairs = 5 if sym else NCLS
    x0_d = nc.dram_tensor("x0", [P, PF], bf16, kind="ExternalInput").ap()
    x1_d = nc.dram_tensor("x1", [P, PF], bf16, kind="ExternalInput").ap()
    wl0_d = nc.dram_tensor("wl0", [P, PF], bf16, kind="ExternalInput").ap()
    wl1_d = nc.dram_tensor("wl1", [P, PF], bf16, kind="ExternalInput").ap()
    wq_d = nc.dram_tensor("wq", [P, 2 * npairs], f32, kind="ExternalInput").ap()
    idp_d = nc.dram_tensor("idp", [P, P], f32, kind="ExternalInput").ap()
    idm_d = nc.dram_tensor("idm", [P, P], f32, kind="ExternalInput").ap()
    out_d = nc.dram_tensor("out", [P, 7], f32, kind="ExternalOutput").ap()

    with tile.TileContext(nc) as tc, ExitStack() as ctx:
        _emit(ctx, tc, nc, sym, x0_d, x1_d, wl0_d, wl1_d, wq_d, idp_d, idm_d, out_d)
    nc.compile()
    return nc


def _emit(ctx, tc, nc, sym, x0_d, x1_d, wl0_d, wl1_d, wq_d, idp_d, idm_d, out_d):
    npairs = 5 if sym else NCLS
    # bufs=2 pools hold tiles whose next-pass write is emitted before this
    # pass's last read (pipelined emission would WAR-deadlock at bufs=1)
    const = ctx.enter_context(tc.tile_pool(name="const", bufs=1))
    dma_p = ctx.enter_context(tc.tile_pool(name="dma", bufs=2))
    h1 = ctx.enter_context(tc.tile_pool(name="h1", bufs=1))
    h2 = ctx.enter_context(tc.tile_pool(name="h2", bufs=2))
    h2b = ctx.enter_context(tc.tile_pool(name="h2b", bufs=1))
    zab = ctx.enter_context(tc.tile_pool(name="zab", bufs=1))
    zcp = ctx.enter_context(tc.tile_pool(name="zcp", bufs=2))
    ep = ctx.enter_context(tc.tile_pool(name="ep", bufs=3))
    etp = ctx.enter_context(tc.tile_pool(name="etp", bufs=2))
    tlA = ctx.enter_context(tc.tile_pool(name="tlA", bufs=2))
    tl = ctx.enter_context(tc.tile_pool(name="tl", bufs=1))
    psum = ctx.enter_context(tc.tile_pool(name="psum", bufs=2, space="PSUM"))

    # one-time constants
    wq = const.tile([P, 2 * npairs], f32, tag="wq")
    nc.sync.dma_start(wq[:], wq_d[:])
    idp = const.tile([P, P], f32r, tag="idp")
    nc.sync.dma_start(idp[:], idp_d[:].bitcast(f32r))
    idm = const.tile([P, P], f32r, tag="idm")
    nc.sync.dma_start(idm[:], idm_d[:].bitcast(f32r))
    sacc = const.tile([P, 7], f32, tag="sacc")

    state = {}

    def h12(t, nchunk=1):
        """DMA + r^2 + 1/r, optionally sub-chunked to shorten the serial
        V->S->V warm-up chain of the very first pass."""
        sl = bass.ts(t, F)
        x0t = dma_p.tile([P, F], bf16, tag="x0t")
        nc.sync.dma_start(x0t[:], x0_d[:, sl])
        x1t = dma_p.tile([P, F], bf16, tag="x1t")
        nc.sync.dma_start(x1t[:], x1_d[:, sl])
        wl0t = dma_p.tile([P, F], bf16, tag="wl0t")
        nc.sync.dma_start(wl0t[:], wl0_d[:, sl])
        wl1t = dma_p.tile([P, F], bf16, tag="wl1t")
        nc.sync.dma_start(wl1t[:], wl1_d[:, sl])

        sq0 = h1.tile([P, F], bf16, tag="sq0")
        sq1 = h1.tile([P, F], bf16, tag="sq1")
        rsq = h2.tile([P, F], bf16, tag="rsq")
        lr = h1.tile([P, F], f32, tag="lr")
        invr = h2.tile([P, F], bf16, tag="invr")
        fc = F // nchunk
        for k in range(nchunk):
            ck = bass.ts(k, fc)
            nc.vector.tensor_mul(sq0[:, ck], x0t[:, ck], x0t[:, ck])
            nc.vector.tensor_mul(sq1[:, ck], x1t[:, ck], x1t[:, ck])
            nc.vector.tensor_add(rsq[:, ck], sq0[:, ck], sq1[:, ck])
            nc.scalar.activation(lr[:, ck], rsq[:, ck], Act.Ln, bias=1e-30)
            nc.scalar.activation(invr[:, ck], lr[:, ck], Act.Exp, scale=-0.5)
        state[t] = dict(x0t=x0t, x1t=x1t, wl0t=wl0t, wl1t=wl1t, rsq=rsq,
                        invr=invr)

    def h3(t):
        """Unit vector, target dot, exp(S*t) + start of the PSUM chain,
        then r/soft accums (kept after tt so ScalarE unblocks early)."""
        st = state[t]
        x0t, x1t, invr, rsq = st["x0t"], st["x1t"], st["invr"], st["rsq"]
        y0 = h2b.tile([P, F], bf16, tag="y0")
        nc.vector.tensor_mul(y0[:], x0t[:], invr[:])
        y1 = h2b.tile([P, F], bf16, tag="y1")
        nc.vector.tensor_mul(y1[:], x1t[:], invr[:])
        t1 = h1.tile([P, F], bf16, tag="t1")
        nc.vector.tensor_mul(t1[:], y0[:], st["wl0t"][:])
        t2 = h1.tile([P, F], bf16, tag="t2")
        nc.vector.tensor_mul(t2[:], y1[:], st["wl1t"][:])
        tt = h2b.tile([P, F], bf16, tag="tt")
        nc.vector.tensor_add(tt[:], t1[:], t2[:])

        e_t = etp.tile([P, F], f32r, tag="et")
        nc.scalar.activation(e_t[:], tt[:], Act.Exp, scale=S)
        es_ps = psum.tile([P, F], f32, tag="es")
        for k in range(F // MM_N):
            ck = bass.ts(k, MM_N)
            nc.tensor.matmul(es_ps[:, ck], idm[:], e_t[:, ck], start=True, stop=False)

        # r and soft partial sums: accum_out = reduce<op1=add>(op0 result):
        # Sa = sum(min(r,1.5)), Sb = sum(max(r,2)); host: Sb - Sa - 0.5*N.
        rs = h1.tile([P, F], bf16, tag="rs")
        nc.vector.tensor_mul(rs[:], rsq[:], invr[:])
        trash_a = h1.tile([P, F], bf16, tag="trash_a")
        nc.vector.tensor_scalar(
            trash_a[:], rs[:], 1.5, None, Alu.min, Alu.add,
            accum_out=sacc[:, 3 * t + 1 : 3 * t + 2],
        )
        trash_b = h1.tile([P, F], bf16, tag="trash_b")
        nc.vector.tensor_scalar(
            trash_b[:], rs[:], 2.0, None, Alu.max, Alu.add,
            accum_out=sacc[:, 3 * t + 2 : 3 * t + 3],
        )
        state[t].update(y0=y0, y1=y1, tt=tt, es=es_ps)

    def tail_pre(t):
        """clip(t), 1-t^2 -- only needs tt; spliced into the class loop."""
        st = state[t]
        tcl = tlA.tile([P, F], bf16, tag="tcl")
        nc.vector.tensor_scalar(tcl[:], st["tt"][:], 1.0, -1.0, Alu.min, Alu.max)
        tsq = tl.tile([P, F], bf16, tag="tsq")
        nc.vector.tensor_mul(tsq[:], tcl[:], tcl[:])
        u = tlA.tile([P, F], bf16, tag="u")
        nc.vector.tensor_scalar(u[:], tsq[:], -1.0, 1.0, Alu.mult, Alu.add)
        state[t].update(tcl=tcl, u=u)

    def classes(t, spliced):
        """Class dot/exp/matmul loop for pass t; `spliced` maps a class
        index to a list of closures emitted after that class (the previous
        pass's tail, interleaved to hide V<->S ping-pong)."""
        st = state[t]
        y0, y1, es_ps = st["y0"], st["y1"], st["es"]
        n_exp = 2 if sym else 1
        for c in range(npairs):
            za = zab.tile([P, F], bf16, tag="za")
            nc.vector.tensor_scalar(
                za[:], y0[:], wq[:, 2 * c : 2 * c + 1], None, Alu.mult
            )
            zb = zab.tile([P, F], bf16, tag="zb")
            nc.vector.tensor_scalar(
                zb[:], y1[:], wq[:, 2 * c + 1 : 2 * c + 2], None, Alu.mult
            )
            zc = zcp.tile([P, F], bf16, tag="zc")
            nc.vector.tensor_add(zc[:], za[:], zb[:])
            for j in range(n_exp):
                e = ep.tile([P, F], f32r, tag="e")
                nc.scalar.activation(e[:], zc[:], Act.Exp, scale=(S, -S)[j])
                last = (c == npairs - 1) and (j == n_exp - 1)
                for k in range(F // MM_N):
                    ck = bass.ts(k, MM_N)
                    nc.tensor.matmul(
                        es_ps[:, ck], idp[:], e[:, ck], start=False, stop=last
                    )
            for fn in spliced.get(c, ()):
                fn()

    def tail_post_ops(t, parts):
        """Stage closures for pass t's tail over free-dim sub-ranges.
        parts: list of (offset, length, lden_col). Returns per-stage lists
        of closures (each stage covers all parts, so a later part's
        ScalarE leg hides an earlier part's VectorE leg and vice versa)."""
        st = state[t]
        # one allocation per tag per pass; parts write disjoint slices
        lnu = tl.tile([P, F], bf16, tag="lnu")
        sqru = tl.tile([P, F], bf16, tag="sqru")
        nump = tl.tile([P, F], bf16, tag="nump")
        e_nn = tl.tile([P, F], f32, tag="enn")
        w = tl.tile([P, F], f32, tag="w")
        trash = tl.tile([P, F], bf16, tag="trash_l")

        def mk(stage, p):
            off, ln, col = parts[p]
            fs = slice(off, off + ln)

            def s_lnu():
                nc.scalar.activation(lnu[:, fs], st["u"][:, fs], Act.Ln, bias=1e-12)
                nc.scalar.activation(
                    sqru[:, fs], lnu[:, fs], Act.Exp, scale=0.5, bias=LN_TAN_M
                )

            def v_nump():
                nc.vector.tensor_tensor(
                    nump[:, fs], st["tcl"][:, fs], sqru[:, fs], Alu.subtract
                )

            def s_enn():
                nc.scalar.activation(
                    e_nn[:, fs], nump[:, fs], Act.Exp, scale=-S * COS_M, bias=-LSH
                )

            def v_w():
                nc.vector.tensor_mul(w[:, fs], st["es"][:, fs], e_nn[:, fs])

            def s_lnacc():
                nc.scalar.activation(
                    trash[:, fs], w[:, fs], Act.Ln, bias=math.exp(-LSH),
                    accum_out=sacc[:, col : col + 1],
                )

            return [s_lnu, v_nump, s_enn, v_w, s_lnacc][stage]

        return [[mk(stage, p) for p in range(len(parts))] for stage in range(5)]

    # ---- software-pipelined emission ----
    h12(0, nchunk=2)
    h12(1)
    h3(0)
    classes(0, {0: [lambda: tail_pre(0)]})
    h3(1)
    spliced = {0: [lambda: tail_pre(1)]}
    stages = tail_post_ops(0, [(0, F, 0)])
    for i, fns in enumerate(stages):
        spliced.setdefault(min(1 + i, npairs - 1), []).extend(fns)
    classes(1, spliced)
    # final tail: two interleaved half-chunks to overlap the V<->S chain
    fstages = tail_post_ops(1, [(0, F // 2, 3), (F // 2, F // 2, 6)])
    for fns in fstages:
        for fn in fns:
            fn()

    nc.sync.dma_start(out_d[:], sacc[:])


_NC_CACHE = {}


def _get_graph(sym: bool):
    if sym not in _NC_CACHE:
        _NC_CACHE[sym] = _build_graph(sym)
    return _NC_CACHE[sym]


def kernel(x, labels, weight):
    import ml_dtypes

    BF = ml_dtypes.bfloat16
    x = np.asarray(x, dtype=np.float32)
    labels = np.asarray(labels).astype(np.int64)
    w = np.asarray(weight, dtype=np.float32)

    wb = w.astype(BF)
    sym = w.shape[0] == NCLS and np.array_equal(
        wb[NCLS // 2 :], -wb[: NCLS // 2]
    )
    npairs = 5 if sym else NCLS
    nc = _get_graph(sym)

    wq = np.ascontiguousarray(
        np.tile(wb[:npairs].astype(np.float32).reshape(1, 2 * npairs), (P, 1))
    )
    idp = np.eye(P, dtype=np.float32)
    idm = -idp

    wl = wb[labels]                      # [N, 2] bf16 gather on host
    x0 = x[:, 0].astype(BF)
    x1 = x[:, 1].astype(BF)

    in_maps = []
    for i in range(N_CORES):
        sl = slice(i * NC_ROWS, (i + 1) * NC_ROWS)
        in_maps.append(
            {
                "x0": np.ascontiguousarray(x0[sl]).reshape(P, PF),
                "x1": np.ascontiguousarray(x1[sl]).reshape(P, PF),
                "wl0": np.ascontiguousarray(wl[sl, 0]).reshape(P, PF),
                "wl1": np.ascontiguousarray(wl[sl, 1]).reshape(P, PF),
                "wq": wq,
                "idp": idp,
                "idm": idm,
            }
        )

    trace = os.environ.get("KTRACE", "0") == "1"
    res = run_bass_kernel_spmd(nc, in_maps, core_ids=list(range(N_CORES)), trace=trace)
    if getattr(res, "exec_time_ns", None):
        print(f"HW exec time: {res.exec_time_ns} ns")

    lden_sum = 0.0
    soft_sum = -0.5 * N
    for i in range(N_CORES):
        o = np.asarray(res.results[i]["out"], dtype=np.float64)
        lden_sum += o[:, 0].sum() + o[:, 3].sum() + o[:, 6].sum()
        soft_sum += (o[:, 2].sum() - o[:, 1].sum()) + (o[:, 5].sum() - o[:, 4].sum())

    loss = lden_sum / N + LSH + (LBDA / 2.0) * (soft_sum / N)
    return np.float32(loss)


if __name__ == "__main__":
    rng = np.random.default_rng(0)
    x = rng.standard_normal((N, 2), dtype=np.float32)
    labels = rng.integers(0, 10, size=(N,)).astype(np.int64)
    w = np.array(
        [[1, 0], [0.809, 0.588], [0.309, 0.951], [-0.309, 0.951], [-0.809, 0.588],
         [-1, 0], [-0.809, -0.588], [-0.309, -0.951], [0.309, -0.951], [0.809, -0.588]],
        dtype=np.float32,
    )
    print(kernel(x, labels, w))
